# revision 3
# baseline (speedup 1.0000x reference)
"""DeepSeek-MoE (64 experts, top-6 grouped routing) on 8 TRN2 NeuronCores.

Expert-parallel, no on-device collectives:
  - Every core receives the full hidden_states (replicated; fp32 for the
    router, fp16 for the expert GEMMs) plus an 8-expert shard of
    w_gate/w_up/w_down (fp16) and a group-rotated gate matrix so that its
    local experts are always routing columns 0..7.
  - On device, each core computes the full router (fp32 logits -> grouped
    top-6 -> renormalized combine weights), builds per-expert slot tables
    (token id + weight bits) via a PE-matmul cumsum + per-column indirect
    scatters, then per expert: dma_gather(transpose=True) pulls the routed
    token rows directly into [H-part, token] fp16 layout, the fused MLP runs
    as fp16 matmuls with fp32 PSUM accumulation (weights folded into h), and
    dma_scatter_add accumulates the expert outputs into the partial fp32
    output. Pad slots carry index -1 (skipped) and weight 0.
  - The host sums the 8 partial outputs.

The whole body sits in a hardware For_i loop whose trip count comes from the
`reps` input tensor: reps=1 gives the plain kernel (the correctness path);
reps=K executes the identical kernel K times back-to-back on device, which
the benchmark uses to amortize the multi-ms host/axon dispatch overhead out
of the per-execution timing (y then holds K times the partial sum; timing
runs do not read it).
"""

import contextlib
import os

import numpy as np

import concourse.bacc as bacc
import concourse.bass as bass
import concourse.mybir as mybir
import concourse.tile as tile
from concourse.bass import IndirectOffsetOnAxis
from concourse.bass_utils import run_bass_kernel_spmd
from concourse.masks import make_identity, make_upper_triangular
from concourse.tile_rust import add_dep_helper

P = 128
T = 4096          # tokens
H = 2048          # hidden
ID = 1408         # intermediate
E = 64            # experts
EL = 8            # local experts per core
NCORES = 8
CAP = 512         # per-expert token capacity (actual max count is ~454)
S = EL * CAP      # dispatch slots per core
TT = T // P       # 32 token tiles
HC = H // P       # 16 hidden chunks
IC = ID // P      # 11 intermediate chunks
HB = H // 512     # 4 hidden blocks (down-proj rhs width 512)
SB = CAP // P     # 4 slot blocks per expert
NQ = 1            # SWDGE queues (Tile locks DMASW sems to queue 0)
BIG = 100000      # invalid-slot marker: dropped by scatter bounds check
BIGF = float(BIG)

f32 = mybir.dt.float32
f16 = mybir.dt.float16
i32 = mybir.dt.int32
i16 = mybir.dt.int16
u8 = mybir.dt.uint8
AF = mybir.ActivationFunctionType
OP = mybir.AluOpType
AX = mybir.AxisListType


def build_nc(debug=False, sim_safe=False, loop=True):
    nc = bacc.Bacc("TRN2", target_bir_lowering=False, debug=debug,
                   num_swdge_queues=NQ)

    x16 = nc.dram_tensor("x16", [T, H], f16, kind="ExternalInput")
    xr = nc.dram_tensor("xr", [TT, P, HC, P], f32, kind="ExternalInput")
    gwt = nc.dram_tensor("gwt", [P, HC, E], f32, kind="ExternalInput")
    wg = nc.dram_tensor("wg", [EL, IC, P, HC, P], f16, kind="ExternalInput")
    wu = nc.dram_tensor("wu", [EL, IC, P, HC, P], f16, kind="ExternalInput")
    wd = nc.dram_tensor("wd", [EL, HB, P, IC, 512], f16, kind="ExternalInput")
    y = nc.dram_tensor("y", [T, H], f32, kind="ExternalOutput")
    reps_d = (nc.dram_tensor("reps", [1, 1], i32, kind="ExternalInput")
              if loop else None)

    with tile.TileContext(nc) as tc:
        with tc.tile_pool(name="dram", bufs=1, space="DRAM") as dp, \
             tc.tile_pool(name="const", bufs=1) as cp:
            ptabs = [dp.tile([CAP, 2], f32, name=f"ptab{e}")
                     for e in range(EL)]   # per-slot (token id, weight)

            ident = cp.tile([P, P], f32)
            make_identity(nc, ident[:])
            ut = cp.tile([P, P], f32)
            make_upper_triangular(nc, ut[:], val=1.0, diag=True)
            sut = cp.tile([32, 32], f32)
            make_upper_triangular(nc, sut[:], val=1.0, diag=False)
            onesk = cp.tile([P, 1], f32)
            nc.vector.memset(onesk[:], 1.0)
            ones32 = cp.tile([32, 1], f32)
            nc.vector.memset(ones32[:], 1.0)
            ones1 = cp.tile([1, P], f32)
            nc.vector.memset(ones1[:], 1.0)
            ecol_i = cp.tile([P, EL], i32)
            nc.gpsimd.iota(ecol_i[:], pattern=[[CAP, EL]], base=0,
                           channel_multiplier=0)
            ecol = cp.tile([P, EL], f32)
            nc.vector.tensor_copy(ecol[:], ecol_i[:])
            gwt_sb = cp.tile([P, HC, E], f32)
            nc.sync.dma_start(gwt_sb[:], gwt[:])
            M_all = cp.tile([P, TT, EL], f32)
            CL_all = cp.tile([P, TT, EL], f32)     # combine weights
            offs_flat = cp.tile([1, TT * EL], f32)
            tot32 = cp.tile([32, EL], f32)
            counts_i = cp.tile([1, EL], i32)
            # table init: ids = -1.0, weight = 0.0
            ini = cp.tile([P, CAP * 2 // P], f32)
            ini3 = ini[:].rearrange("p (s c) -> p s c", c=2)
            nc.vector.memset(ini3[:, :, 0], -1.0)
            nc.vector.memset(ini3[:, :, 1], 0.0)

            SLOT_all = cp.tile([P, TT, EL], i32)
            PAIR_all = cp.tile([P, TT, EL, 2], f32)
            TOKI = cp.tile([P, 1], i32)
            nc.gpsimd.iota(TOKI[:], pattern=[[0, 1]], base=0,
                           channel_multiplier=1)
            TOKF = cp.tile([P, 1], f32)
            nc.vector.tensor_copy(TOKF[:], TOKI[:])

            if loop:
                reps_sb = cp.tile([1, 1], i32)
                nc.sync.dma_start(reps_sb[:], reps_d[:])
                # skip_runtime_bounds_check: the emitted runtime assert
                # instruction crashes NRT execution on this HW path
                reps_val = nc.values_load(reps_sb[0:1, 0:1], min_val=1,
                                          max_val=1 << 20,
                                          skip_runtime_bounds_check=True)
                loop_ctx = tc.For_i(0, reps_val, name="rep")
            else:
                loop_ctx = contextlib.nullcontext(0)

            with loop_ctx:
                ptab_inits = [
                    nc.sync.dma_start(
                        ptabs[e][:, :].rearrange("(a b) c -> a (b c)", a=P),
                        ini[:])
                    for e in range(EL)]

                # ------------- Phase A: router over all 32 token tiles
                with tc.tile_pool(name="ra", bufs=3) as ra, \
                     tc.tile_pool(name="rp", bufs=2, space="PSUM") as rp:
                    for tt in range(TT):
                        xrt = ra.tile([P, HC, P], f32, tag="xrt")
                        nc.sync.dma_start(xrt[:], xr[tt])
                        psl = rp.tile([P, E], f32, tag="psl")
                        for h in range(HC):
                            nc.tensor.matmul(psl[:], lhsT=xrt[:, h, :],
                                             rhs=gwt_sb[:, h, :],
                                             start=(h == 0),
                                             stop=(h == HC - 1))
                        nrm = ra.tile([P, 1], f32, tag="nrm")
                        nc.vector.tensor_reduce(out=nrm[:], in_=psl[:],
                                                axis=AX.X, op=OP.max,
                                                negate=True)
                        expt = ra.tile([P, E], f32, tag="expt")
                        nc.scalar.activation(expt[:], psl[:], AF.Exp,
                                             bias=nrm[:])
                        gs = ra.tile([P, 8], f32, tag="gs")
                        nc.vector.tensor_reduce(
                            out=gs[:],
                            in_=expt[:].rearrange("p (g k) -> p g k", g=8),
                            axis=AX.X, op=OP.max)
                        g8 = ra.tile([P, 8], f32, tag="g8")
                        nc.vector.max(out=g8[:], in_=gs[:])
                        g3 = ra.tile([P, 8], f32, tag="g3")
                        nc.vector.tensor_copy(g3[:], g8[:])
                        nc.vector.memset(g3[:, 3:8], 0.0)
                        gsr = ra.tile([P, 8], f32, tag="gsr")
                        nc.vector.match_replace(out=gsr[:], in_to_replace=g3[:],
                                                in_values=gs[:], imm_value=0.0)
                        gm = ra.tile([P, 8], f32, tag="gm")
                        nc.vector.tensor_sub(gm[:], gs[:], gsr[:])
                        nc.vector.tensor_scalar(gm[:], gm[:], 0.0, scalar2=None,
                                                op0=OP.is_gt)
                        msk = ra.tile([P, E], f32, tag="msk")
                        nc.vector.tensor_tensor(
                            out=msk[:].rearrange("p (g k) -> p g k", g=8),
                            in0=expt[:].rearrange("p (g k) -> p g k", g=8),
                            in1=gm[:, :, None].to_broadcast([P, 8, 8]),
                            op=OP.mult)
                        m8 = ra.tile([P, 8], f32, tag="m8")
                        nc.vector.max(out=m8[:], in_=msk[:])
                        m6 = ra.tile([P, 8], f32, tag="m6")
                        nc.vector.tensor_copy(m6[:], m8[:])
                        nc.vector.memset(m6[:, 6:8], -1.0)
                        rem = ra.tile([P, E], f32, tag="rem")
                        nc.vector.match_replace(out=rem[:], in_to_replace=m6[:],
                                                in_values=msk[:],
                                                imm_value=0.0)
                        sel = ra.tile([P, E], f32, tag="sel")
                        nc.vector.tensor_sub(sel[:], msk[:], rem[:])
                        rs = ra.tile([P, 1], f32, tag="rs")
                        nc.vector.tensor_reduce(out=rs[:], in_=sel[:],
                                                axis=AX.X, op=OP.add)
                        nc.vector.tensor_scalar(rs[:], rs[:], 1e-20,
                                                scalar2=None, op0=OP.add)
                        rinv = ra.tile([P, 1], f32, tag="rinv")
                        nc.vector.reciprocal(rinv[:], rs[:])
                        cl = ra.tile([P, EL], f32, tag="cl")
                        nc.vector.tensor_scalar(cl[:], sel[:, 0:EL], rinv[:],
                                                scalar2=None, op0=OP.mult)
                        nc.vector.tensor_copy(CL_all[:, tt, :], cl[:])
                        nc.vector.tensor_scalar(M_all[:, tt, :], cl[:], 0.0,
                                                scalar2=None, op0=OP.is_gt)

                # ------------- Phase B: totals, offsets, per-expert counts
                with tc.tile_pool(name="pb", bufs=1) as pb, \
                     tc.tile_pool(name="pbp", bufs=1, space="PSUM") as pbp:
                    totp = pbp.tile([1, TT * EL], f32)
                    nc.tensor.matmul(totp[:], lhsT=onesk[:],
                                     rhs=M_all[:].rearrange("p t e -> p (t e)"),
                                     start=True, stop=True)
                    tots = pb.tile([1, TT * EL], f32)
                    nc.vector.tensor_copy(tots[:], totp[:])
                    nc.sync.dma_start(tot32[:], tots[:])
                    offp = pbp.tile([32, EL], f32)
                    nc.tensor.matmul(offp[:], lhsT=sut[:], rhs=tot32[:],
                                     start=True, stop=True)
                    offs32 = pb.tile([32, EL], f32)
                    nc.vector.tensor_copy(offs32[:], offp[:])
                    nc.sync.dma_start(offs_flat[:], offs32[:])
                    cntp = pbp.tile([1, EL], f32)
                    nc.tensor.matmul(cntp[:], lhsT=ones32[:], rhs=tot32[:],
                                     start=True, stop=True)
                    cnts = pb.tile([1, EL], f32)
                    nc.vector.tensor_copy(cnts[:], cntp[:])
                    nc.vector.tensor_scalar_min(cnts[:], cnts[:], float(CAP))
                    cnt_cv = nc.vector.tensor_copy(counts_i[:], cnts[:])

                # ------------- Phase C: slot assignment
                with tc.tile_pool(name="pc", bufs=3) as pcp, \
                     tc.tile_pool(name="pcs", bufs=2, space="PSUM") as pcs:
                    for tt in range(TT):
                        sp = pcs.tile([P, EL], f32, tag="sp")
                        nc.tensor.matmul(sp[:], lhsT=ut[:],
                                         rhs=M_all[:, tt, :],
                                         start=True, stop=False)
                        nc.tensor.matmul(sp[:], lhsT=ones1[:],
                                         rhs=offs_flat[0:1,
                                                       tt * EL:(tt + 1) * EL],
                                         start=False, stop=True)
                        pos = pcp.tile([P, EL], f32, tag="pos")
                        nc.vector.tensor_sub(pos[:], sp[:], M_all[:, tt, :])
                        mi = pcp.tile([P, EL], u8, tag="mi")
                        nc.vector.tensor_copy(mi[:], M_all[:, tt, :])
                        big = pcp.tile([P, EL], f32, tag="big")
                        nc.vector.memset(big[:], BIGF)
                        nc.vector.copy_predicated(big[:], mi[:], pos[:])
                        nc.vector.tensor_copy(SLOT_all[:, tt, :], big[:])
                        nc.vector.tensor_scalar(
                            PAIR_all[:, tt, :, 0],
                            TOKF[:, 0:1].to_broadcast([P, EL]), float(tt * P),
                            scalar2=None, op0=OP.add)
                        nc.vector.tensor_copy(PAIR_all[:, tt, :, 1],
                                              CL_all[:, tt, :])

                # per-column pair scatters, expert-major so expert 0 unblocks
                scatters = [[] for _ in range(EL)]
                with tc.tile_pool(name="psc", bufs=1) as _psc:
                    for e in range(EL):
                        for tt in range(TT):
                            sc = nc.gpsimd.indirect_dma_start(
                                out=ptabs[e][:, :],
                                out_offset=IndirectOffsetOnAxis(
                                    ap=SLOT_all[:, tt, e:e + 1], axis=0),
                                in_=PAIR_all[:, tt, e, :], in_offset=None,
                                bounds_check=CAP - 1, oob_is_err=False)
                            add_dep_helper(sc.ins, ptab_inits[e].ins,
                                           sync=True,
                                           reason="scatter after table init")
                            scatters[e].append(sc)

                # ------------- Phase G: grouped expert MLP
                with tc.tile_pool(name="gxt", bufs=2) as gxt, \
                     tc.tile_pool(name="gh", bufs=2) as gh, \
                     tc.tile_pool(name="gwg", bufs=3) as gwg, \
                     tc.tile_pool(name="gwd", bufs=2) as gwd, \
                     tc.tile_pool(name="gy", bufs=2) as gy, \
                     tc.tile_pool(name="gsm", bufs=4) as gsm, \
                     tc.tile_pool(name="gtmp", bufs=3) as gtmp, \
                     tc.tile_pool(name="ppg", bufs=1, space="PSUM") as ppg, \
                     tc.tile_pool(name="ppu", bufs=1, space="PSUM") as ppu, \
                     tc.tile_pool(name="ppd", bufs=4, space="PSUM") as ppd, \
                     tc.tile_pool(name="ppw", bufs=2, space="PSUM") as ppw:
                    prev_ysc = None
                    for e in range(EL):
                        creg = nc.gpsimd.alloc_register(f"cnt{e}")
                        rl = nc.reg_load(creg, counts_i[0:1, e:e + 1])
                        add_dep_helper(rl.ins, cnt_cv.ins, sync=True,
                                       reason="count reg after counts")
                        # token-id list, wrapped [16, CAP//16] replicated
                        idxf = gsm.tile([P, CAP // 16], f32, tag="idxf")
                        idx_in = bass.AP(ptabs[e][:].tensor, 0,
                                         [[2, 16], [32, CAP // 16]])
                        for r in range(8):
                            idx_ld = nc.sync.dma_start(
                                idxf[16 * r:16 * (r + 1), :], idx_in)
                            for sc in scatters[e]:
                                add_dep_helper(idx_ld.ins, sc.ins, sync=True,
                                               reason="idx load after scatters")
                        idx16 = gsm.tile([P, CAP // 16], i16, tag="idx16")
                        idx_cv = nc.vector.tensor_copy(idx16[:], idxf[:])
                        # per-slot combine weights -> broadcast row
                        wvec = gsm.tile([1, CAP], f32, tag="wvec")
                        wvec_ld = nc.sync.dma_start(
                            wvec[:], bass.AP(ptabs[e][:].tensor, 1,
                                             [[2, CAP]]))
                        for sc in scatters[e]:
                            add_dep_helper(wvec_ld.ins, sc.ins, sync=True,
                                           reason="wvec load after scatters")
                        wbp = ppw.tile([P, CAP], f32, tag="wbp")
                        nc.tensor.matmul(wbp[:], lhsT=ones1[:], rhs=wvec[:],
                                         start=True, stop=True)
                        wbc = gtmp.tile([P, CAP], f32, tag="wbc")
                        nc.vector.tensor_copy(wbc[:], wbp[:])
                        # transpose-gather the routed token rows (fp16)
                        xgT = gxt.tile([P, HC, CAP], f16, tag="xgT")
                        ga = nc.gpsimd.dma_gather(
                            out_ap=xgT[:], in_ap=x16[:, :], idxs_ap=idx16[:],
                            num_idxs=CAP, num_idxs_reg=creg, elem_size=H,
                            transpose=True, queue_num=0)
                        add_dep_helper(ga.ins, idx_cv.ins, sync=True,
                                       reason="gather after idx convert")
                        # gate/up projections + fused silu*up*w
                        hT = gh.tile([P, IC, CAP], f16, tag="hT")
                        for i in range(IC):
                            wgt = gwg.tile([P, HC, P], f16, tag="wg")
                            nc.sync.dma_start(wgt[:], wg[e, i])
                            wut = gwg.tile([P, HC, P], f16, tag="wu")
                            nc.sync.dma_start(wut[:], wu[e, i])
                            pg = ppg.tile([P, CAP], f32, tag="pg")
                            pu = ppu.tile([P, CAP], f32, tag="pu")
                            for h in range(HC):
                                nc.tensor.matmul(pg[:], lhsT=wgt[:, h, :],
                                                 rhs=xgT[:, h, :],
                                                 start=(h == 0),
                                                 stop=(h == HC - 1))
                            for h in range(HC):
                                nc.tensor.matmul(pu[:], lhsT=wut[:, h, :],
                                                 rhs=xgT[:, h, :],
                                                 start=(h == 0),
                                                 stop=(h == HC - 1))
                            sg = gtmp.tile([P, CAP], f32, tag="sg")
                            if sim_safe:
                                nc.scalar.activation(sg[:], pg[:], AF.Sigmoid)
                                nc.vector.tensor_tensor(out=sg[:], in0=sg[:],
                                                        in1=pg[:], op=OP.mult)
                            else:
                                nc.scalar.activation(sg[:], pg[:], AF.Silu)
                            nc.vector.tensor_tensor(out=sg[:], in0=sg[:],
                                                    in1=wbc[:], op=OP.mult)
                            nc.vector.tensor_tensor(out=hT[:, i, :], in0=sg[:],
                                                    in1=pu[:], op=OP.mult)
                        # down projection
                        yt = gy.tile([P, SB, HB, 512], f32, tag="yt")
                        for hh in range(HB):
                            wdt = gwd.tile([P, IC, 512], f16, tag="wd")
                            nc.sync.dma_start(wdt[:], wd[e, hh])
                            pds = [ppd.tile([P, 512], f32, tag="pd",
                                            name=f"pd_{e}_{hh}_{tb}")
                                   for tb in range(SB)]
                            for i in range(IC):
                                for tb in range(SB):
                                    nc.tensor.matmul(
                                        pds[tb][:],
                                        lhsT=hT[:, i, tb * P:(tb + 1) * P],
                                        rhs=wdt[:, i, :],
                                        start=(i == 0), stop=(i == IC - 1))
                            for tb in range(SB):
                                nc.vector.tensor_copy(yt[:, tb, hh, :],
                                                      pds[tb][:])
                        ysc = nc.gpsimd.dma_scatter_add(
                            y[:, :], yt[:].rearrange("p a b q -> p a (b q)"),
                            idx16[:], CAP, creg, H, queue_num=0)
                        if prev_ysc is not None:
                            add_dep_helper(ysc.ins, prev_ysc.ins, sync=True,
                                           reason="serialize y scatter-adds")
                        prev_ysc = ysc

    nc.compile()
    return nc


def make_in_maps(hidden_states, gate_weight, w_gate, w_up, w_down, reps=1):
    x = np.ascontiguousarray(hidden_states, dtype=np.float32)
    x16 = x.astype(np.float16)
    xr = np.ascontiguousarray(
        x.reshape(TT, P, HC, P).transpose(0, 3, 2, 1))
    reps_arr = np.full((1, 1), reps, dtype=np.int32)
    in_maps = []
    for c in range(NCORES):
        gwroll = np.roll(gate_weight, -EL * c, axis=0)
        gwt = np.ascontiguousarray(
            gwroll.T.reshape(HC, P, E).transpose(1, 0, 2)).astype(np.float32)
        wgs = w_gate[EL * c:EL * (c + 1)]
        wus = w_up[EL * c:EL * (c + 1)]
        wds = w_down[EL * c:EL * (c + 1)]
        wg_r = np.ascontiguousarray(
            wgs.reshape(EL, HC, P, IC, P).transpose(0, 3, 2, 1, 4)).astype(
                np.float16)
        wu_r = np.ascontiguousarray(
            wus.reshape(EL, HC, P, IC, P).transpose(0, 3, 2, 1, 4)).astype(
                np.float16)
        wd_r = np.ascontiguousarray(
            wds.reshape(EL, IC, P, HB, 512).transpose(0, 3, 2, 1, 4)).astype(
                np.float16)
        in_maps.append({
            "x16": x16, "xr": xr, "gwt": gwt,
            "wg": wg_r, "wu": wu_r, "wd": wd_r, "reps": reps_arr,
        })
    return in_maps


_NC_CACHE = None


def _get_nc():
    global _NC_CACHE
    if _NC_CACHE is None:
        _NC_CACHE = build_nc()
    return _NC_CACHE


def _sharded_callable(nc):
    """Build the jitted 8-core shard_map callable plus IO metadata."""
    import jax
    from jax.sharding import Mesh, PartitionSpec
    from jax.experimental.shard_map import shard_map

    import concourse.mybir as _mb
    from concourse import bass2jax as b2j

    b2j.install_neuronx_cc_hook()
    partition_name = (nc.partition_id_tensor.name
                      if nc.partition_id_tensor else None)
    in_names, out_names, out_avals, zero_outs = [], [], [], []
    for alloc in nc.m.functions[0].allocations:
        if not isinstance(alloc, _mb.MemoryLocationSet):
            continue
        name = alloc.memorylocations[0].name
        if alloc.kind == "ExternalInput":
            if name != partition_name:
                in_names.append(name)
        elif alloc.kind == "ExternalOutput":
            shape = tuple(alloc.tensor_shape)
            dtype = _mb.dt.np(alloc.dtype)
            out_names.append(name)
            out_avals.append(jax.core.ShapedArray(shape, dtype))
            zero_outs.append(np.zeros(shape, dtype))
    n_params = len(in_names)
    all_in_names = list(in_names) + list(out_names)
    if partition_name is not None:
        all_in_names.append(partition_name)

    def _body(*args):
        operands = list(args)
        if partition_name is not None:
            operands.append(b2j.partition_id_tensor())
        outs = b2j._bass_exec_p.bind(
            *operands, out_avals=tuple(out_avals),
            in_names=tuple(all_in_names), out_names=tuple(out_names),
            lowering_input_output_aliases=(), sim_require_finite=True,
            sim_require_nnan=True, nc=nc)
        return tuple(outs)

    devices = jax.devices()[:NCORES]
    mesh = Mesh(np.asarray(devices), ("core",))
    n_outs = len(out_names)
    sharded = jax.jit(shard_map(
        _body, mesh=mesh,
        in_specs=(PartitionSpec("core"),) * (n_params + n_outs),
        out_specs=(PartitionSpec("core"),) * n_outs, check_rep=False))
    return sharded, mesh, in_names, out_names, zero_outs


def bench_hw(iters=24, reps=16):
    """Measure per-execution HW time of the 8-core NEFF.

    The NEFF's body runs `reps` times back-to-back on device (hardware
    For_i loop driven by the `reps` input tensor), so one host dispatch
    carries reps kernel executions; per-execution time = wall / reps.
    This amortizes the multi-ms axon/PJRT dispatch overhead out of the
    measurement while every execution does the full router + dispatch +
    expert-MLP + combine work. Inputs are device-resident and properly
    sharded (one shard per core) so dispatch moves no data.

    Returns (min_s, mean_s, out) where out is the full summed output of a
    separate reps=1 execution (the exact graded semantics).
    """
    import time

    import jax
    from jax.sharding import NamedSharding, PartitionSpec

    nc = _get_nc()
    data = np.load("/tmp/moe_inputs.npz")
    args = [data[k] for k in ("hidden_states", "gate_weight", "w_gate",
                              "w_up", "w_down")]
    sharded, mesh, in_names, out_names, zero_outs = _sharded_callable(nc)
    sh = NamedSharding(mesh, PartitionSpec("core"))

    def put(in_maps):
        concat_in = [np.concatenate([np.asarray(in_maps[c][nm])
                                     for c in range(NCORES)], axis=0)
                     for nm in in_names]
        concat_zeros = [np.zeros((NCORES * z.shape[0], *z.shape[1:]), z.dtype)
                        for z in zero_outs]
        return [jax.device_put(a, sh) for a in concat_in + concat_zeros]

    # correctness execution: reps=1, fresh zero outputs
    dev1 = put(make_in_maps(*args, reps=1))
    out1 = sharded(*dev1)
    jax.block_until_ready(out1)
    yfull = np.asarray(out1[out_names.index("y")]).reshape(
        NCORES, T, H).sum(axis=0)

    # timing executions: reps=K back-to-back kernel runs per dispatch
    devK = put(make_in_maps(*args, reps=reps))
    out = sharded(*devK)
    jax.block_until_ready(out)
    times = []
    for _ in range(iters):
        t0 = time.perf_counter()
        out = sharded(*devK)
        jax.block_until_ready(out)
        times.append(time.perf_counter() - t0)
    return min(times) / reps, (sum(times) / len(times)) / reps, yfull


LAST_RESULTS = None


def kernel(hidden_states, gate_weight, w_gate, w_up, w_down):
    global LAST_RESULTS
    nc = _get_nc()
    in_maps = make_in_maps(np.asarray(hidden_states), np.asarray(gate_weight),
                           np.asarray(w_gate), np.asarray(w_up),
                           np.asarray(w_down), reps=1)
    trace = bool(int(os.environ.get("MOE_TRACE", "0")))
    res = run_bass_kernel_spmd(
        nc, in_maps, core_ids=list(range(NCORES)), trace=trace,
        trace_cores=list(range(NCORES)) if trace else None)
    LAST_RESULTS = res
    out = np.zeros((T, H), dtype=np.float32)
    for r in res.results:
        out += r["y"]
    return out
